# revision 3
# baseline (speedup 1.0000x reference)
"""AFT-Full attention kernel for 8 TRN2 NeuronCores.

Math: the reference's exp_pos_bias = exp(pos_bias - max(pos_bias, axis=0)) is
identically 1.0 (the max is over a singleton dim), so the two (b,Ti,Tj,Dh)
einsums collapse to per-(b,h) sums over j:
    num[b,h] = sum_j exp(K-max_b K)[b,j,h] * V[b,j,h]
    den[b,h] = sum_j exp(K-max_b K)[b,j,h]
    out = (sigmoid(Q) * num/den) @ Wo.T

Sharding: sequence-parallel over T (256 positions per core, all 4 batches),
weights replicated.  Per-core row index r = b*256 + t_local (b-major) so the
per-(b,h) sums over t are contiguous innermost reduces.  One 32 KiB AllReduce
combines the per-core partial num/den; a 4-byte dummy AllReduce issued at
kernel start absorbs the ~60us one-time collective channel setup so the real
one (~12us) hides behind the Q projection.
"""

import numpy as np
import ml_dtypes

import concourse.bass as bass
import concourse.mybir as mybir
import concourse.tile as tile
from concourse import bacc
from concourse.bass_utils import run_bass_kernel_spmd

B, T, DM, DH = 4, 2048, 1024, 1024
N_CORES = 8
TC = T // N_CORES          # 256 sequence positions per core
R = B * TC                 # 1024 rows per core, r = b*256 + t
P = 128
MC = DM // P               # 8 contraction chunks (d_model)
HB = DH // P               # 8 hidden blocks
NB = 512                   # matmul moving free dim
RB = R // NB               # 2 row blocks
MB = DM // NB              # 2 output-model blocks

F16 = mybir.dt.bfloat16
F32 = mybir.dt.float32
NPF16 = ml_dtypes.bfloat16

_GRAPH = None


def _body(nc, tc):
    qT = nc.dram_tensor("qT", [DM, R], F16, kind="ExternalInput").ap()
    kT = nc.dram_tensor("kT", [DM, R], F16, kind="ExternalInput").ap()
    vT = nc.dram_tensor("vT", [DM, R], F16, kind="ExternalInput").ap()
    wqT = nc.dram_tensor("wqT", [DM, DH], F16, kind="ExternalInput").ap()
    wkT = nc.dram_tensor("wkT", [DM, DH], F16, kind="ExternalInput").ap()
    wvT = nc.dram_tensor("wvT", [DM, DH], F16, kind="ExternalInput").ap()
    woT = nc.dram_tensor("woT", [DH, DM], F16, kind="ExternalInput").ap()
    out = nc.dram_tensor("out", [R, DM], F32, kind="ExternalOutput").ap()

    Exp = mybir.ActivationFunctionType.Exp
    Sigmoid = mybir.ActivationFunctionType.Sigmoid
    Op = mybir.AluOpType

    from contextlib import ExitStack
    with ExitStack() as ctx:
        acts = ctx.enter_context(tc.tile_pool(name="acts", bufs=1))
        work = ctx.enter_context(tc.tile_pool(name="work", bufs=2))
        sqp = ctx.enter_context(tc.tile_pool(name="sqp", bufs=1))
        psum = ctx.enter_context(tc.tile_pool(name="psum", bufs=2, space="PSUM"))
        dram = ctx.enter_context(tc.tile_pool(name="dram", bufs=1, space="DRAM"))

        # Dummy 4-byte AllReduce issued first: the first collective doorbell
        # pays a ~60us one-time channel setup; absorbing it here (concurrent
        # with the DMA ramp and K/V phase) makes the real AllReduce ~12us.
        # The warmup matmuls on zeros un-throttle the PE clock while the
        # first input chunks stream in; their result (0.0) feeds the dummy
        # collective so they are not dead code.
        warm = acts.tile([P, 640], F16, name="warm")
        nc.gpsimd.memset(warm[:], 0.0)
        # alternate two psum tiles so warmups pipeline instead of
        # WAW-serializing; they must end by first-data arrival (~13.4us)
        pwu = psum.tile([P, NB], F32, name="pwu", tag="pk")
        pwu2 = psum.tile([P, NB], F32, name="pwu2", tag="pv")
        for i in range(16):
            t = pwu if i % 2 == 0 else pwu2
            nc.tensor.matmul(t[:], warm[:, 0:P], warm[:, P:P + NB],
                             start=True, stop=True)
        dummy = acts.tile([1, 1], F32, name="dummy")
        nc.vector.tensor_copy(dummy[:], pwu[0:1, 0:1])
        nc.vector.tensor_copy(dummy[:], pwu2[0:1, 0:1])
        d_in = dram.tile([1, 1], F32, name="d_in")
        d_out = dram.tile([1, 1], F32, name="d_out", addr_space="Shared")
        nc.sync.dma_start(d_in[:], dummy[:])
        nc.gpsimd.collective_compute(
            "AllReduce", mybir.AluOpType.add,
            replica_groups=[list(range(N_CORES))],
            ins=[d_in.opt()], outs=[d_out.opt()],
        )

        def declare(name, free):
            return acts.tile([P, MC, free], F16, name=name)

        def load_chunk(t, ap_dram, mc):
            src = ap_dram.rearrange("(c p) f -> p c f", p=P)
            nc.sync.dma_start(t[:, mc, :], src[:, mc, :])

        def load(ap_dram, name, free):
            t = declare(name, free)
            nc.sync.dma_start(t[:], ap_dram.rearrange("(c p) f -> p c f", p=P))
            return t

        # K/V operands stream in per-mc so the first matmuls start early
        # (sync ring drains in order).
        kt = declare("kt", R)
        wk = declare("wk", DH)
        for mc in range(MC):
            load_chunk(wk, wkT, mc)
            load_chunk(kt, kT, mc)
        vt = declare("vt", R)
        wv = declare("wv", DH)
        for mc in range(MC):
            load_chunk(wv, wvT, mc)
            load_chunk(vt, vT, mc)
        qt = load(qT, "qt", R)
        wq = load(wqT, "wq", DH)
        wo = load(woT, "wo", DM)

        nd_all = acts.tile([P, HB, 8], F32, name="nd_all")

        # ---- K/V projections + exp + partial num/den ----
        # K runs one hb ahead of V: kt/wk arrive first, and the PE clock is
        # un-throttled only for the first ~24us, so front-load K matmuls.
        def kproj(hb):
            hs = slice(hb * P, (hb + 1) * P)
            pk = psum.tile([P, R], F32, name="pk", tag="pk")
            for mc in range(MC):
                for rb in range(RB):
                    nc.tensor.matmul(
                        pk[:, rb * NB:(rb + 1) * NB],
                        wk[:, mc, hs],
                        kt[:, mc, rb * NB:(rb + 1) * NB],
                        start=(mc == 0), stop=(mc == MC - 1),
                    )
            return pk

        pk_next = kproj(0)
        for hb in range(HB):
            hs = slice(hb * P, (hb + 1) * P)
            pk = pk_next
            if hb + 1 < HB:
                pk_next = kproj(hb + 1)
            pv = psum.tile([P, R], F32, name="pv", tag="pv")
            for mc in range(MC):
                for rb in range(RB):
                    nc.tensor.matmul(
                        pv[:, rb * NB:(rb + 1) * NB],
                        wv[:, mc, hs],
                        vt[:, mc, rb * NB:(rb + 1) * NB],
                        start=(mc == 0), stop=(mc == MC - 1),
                    )
            # max over batch: copy + 3 maxes (DVE may read PSUM only once/op)
            mk = work.tile([P, TC], F32, name="mk")
            nc.vector.tensor_copy(mk[:], pk[:, 0 * TC:1 * TC])
            for b in range(1, B):
                nc.vector.tensor_tensor(
                    mk[:], mk[:], pk[:, b * TC:(b + 1) * TC], op=Op.max)
            ek = work.tile([P, R], F32, name="ek")
            ek3 = ek.rearrange("p (b t) -> p b t", t=TC)
            nc.vector.tensor_tensor(
                ek3, pk.rearrange("p (b t) -> p b t", t=TC),
                mk[:, None, :].to_broadcast((P, B, TC)), op=Op.subtract)
            # exp on ACT with fused per-b den accumulation (contiguous slices)
            for b in range(B):
                bs = slice(b * TC, (b + 1) * TC)
                nc.scalar.activation(
                    ek[:, bs], ek[:, bs], Exp,
                    accum_out=nd_all[:, hb, b:b + 1])
            ekv = work.tile([P, R], F32, name="ekv")
            nc.vector.tensor_tensor(ekv[:], ek[:], pv[:], op=Op.mult)
            nc.vector.tensor_reduce(
                nd_all[:, hb, B:2 * B], ekv.rearrange("p (b t) -> p b t", t=TC),
                axis=mybir.AxisListType.X, op=Op.add)

        # ---- AllReduce of partial num/den (32 KiB) ----
        red_in = dram.tile([P, HB * 8], F32, name="red_in")
        red_out = dram.tile([P, HB * 8], F32, name="red_out", addr_space="Shared")
        nc.sync.dma_start(red_in[:], nd_all[:])
        nc.gpsimd.collective_compute(
            "AllReduce", Op.add,
            replica_groups=[list(range(N_CORES))],
            ins=[red_in.opt()], outs=[red_out.opt()],
        )
        nd_sum = acts.tile([P, HB, 8], F32, name="nd_sum")
        nc.sync.dma_start(nd_sum[:], red_out[:])

        # ---- Q projection + sigmoid (overlaps the collective) ----
        sq = []
        for hb in range(HB):
            hs = slice(hb * P, (hb + 1) * P)
            pq = psum.tile([P, R], F32, name="pq", tag="pk")
            for mc in range(MC):
                for rb in range(RB):
                    nc.tensor.matmul(
                        pq[:, rb * NB:(rb + 1) * NB],
                        wq[:, mc, hs],
                        qt[:, mc, rb * NB:(rb + 1) * NB],
                        start=(mc == 0), stop=(mc == MC - 1),
                    )
            s = sqp.tile([P, R], F16, name=f"sq{hb}")
            nc.scalar.activation(s[:], pq[:], Sigmoid)
            sq.append(s)

        # ---- r = num/den, yt = sigmoid(Q) * r ----
        rden = acts.tile([P, HB, B], F32, name="rden")
        nc.vector.reciprocal(rden[:], nd_sum[:, :, 0:B])
        r_bf = acts.tile([P, HB, B], F16, name="r_bf")
        nc.vector.tensor_tensor(r_bf[:], nd_sum[:, :, B:2 * B], rden[:], op=Op.mult)

        # ---- output projection (yt = sig*r folded in just before first use
        # so the first O matmuls don't wait for the whole yt chain) ----
        for rblk in range(HB):
            rs = slice(rblk * P, (rblk + 1) * P)
            po = psum.tile([P, DM], F32, name="po", tag="pv")
            for hc in range(HB):
                if rblk == 0:
                    s3 = sq[hc].rearrange("p (b t) -> p b t", t=TC)
                    nc.vector.tensor_tensor(
                        s3, s3, r_bf[:, hc, :, None].to_broadcast((P, B, TC)),
                        op=Op.mult)
                for mb in range(MB):
                    nc.tensor.matmul(
                        po[:, mb * NB:(mb + 1) * NB],
                        sq[hc][:, rs],
                        wo[:, hc, mb * NB:(mb + 1) * NB],
                        start=(hc == 0), stop=(hc == HB - 1),
                    )
            ot = work.tile([P, DM], F32, name="ot")
            if rblk == HB - 1:
                # split the last block so copy/DMA pipeline after the final
                # matmul instead of serializing one big copy + DMA
                nc.vector.tensor_copy(ot[:, 0:NB], po[:, 0:NB])
                nc.sync.dma_start(out[rs, 0:NB], ot[:, 0:NB])
                nc.scalar.copy(ot[:, NB:DM], po[:, NB:DM])
                nc.sync.dma_start(out[rs, NB:DM], ot[:, NB:DM])
            else:
                if rblk % 2 == 0:
                    nc.vector.tensor_copy(ot[:], po[:])
                else:
                    nc.scalar.copy(ot[:], po[:])
                nc.sync.dma_start(out[rs, :], ot[:])


def _dedup_ldweights(nc):
    """Drop InstLdweights whose weight AP is identical to the PE's already-
    loaded weights (the 2nd matmul of each rb-pair reloads the same tile).
    Each reload costs ~50ns of weight-plane fill serialized into the next
    matmul (263ns vs 216ns per MM measured), so halving LDW count saves
    ~12-25us across 512 matmuls.  Waits/updates of a dropped LDW are merged
    into the following PE instruction (fires later -> still safe)."""
    PE = mybir.EngineType.PE
    for f in nc.m.functions:
        for blk in f.blocks:
            insts = list(blk.instructions)
            keep = []
            last_sig = None
            pending_si = None
            for inst in insts:
                if inst.engine == PE:
                    if isinstance(inst, mybir.InstLdweights):
                        sig = (str(inst.ins[0]), str(inst.tile_position),
                               str(inst.perf_mode), str(inst.is_transpose))
                        if sig == last_sig:
                            si = inst.sync_info
                            if si is not None and (si.on_wait or si.on_update):
                                if pending_si is None:
                                    pending_si = ([], [])
                                pending_si[0].extend(si.on_wait)
                                pending_si[1].extend(si.on_update)
                            continue  # drop redundant reload
                        last_sig = sig
                    elif isinstance(inst, mybir.InstMatmult):
                        if pending_si is not None:
                            si = inst.sync_info
                            if si is None:
                                si = mybir.SyncInfo(on_wait=[], on_update=[])
                            inst.sync_info = mybir.SyncInfo(
                                on_wait=list(si.on_wait) + pending_si[0],
                                on_update=list(si.on_update) + pending_si[1],
                            )
                            pending_si = None
                    else:
                        last_sig = None  # unknown PE op: be conservative
                keep.append(inst)
            assert pending_si is None
            if len(keep) != len(insts):
                blk.instructions[:] = keep


def _build():
    global _GRAPH
    if _GRAPH is None:
        nc = bacc.Bacc("TRN2", target_bir_lowering=False, debug=False,
                       num_devices=N_CORES)
        with tile.TileContext(nc) as tc:
            _body(nc, tc)
        _dedup_ldweights(nc)
        nc.compile()
        _GRAPH = nc
    return _GRAPH


def _shard_inputs(inputs):
    q = np.asarray(inputs["q"], np.float32)
    k = np.asarray(inputs["k"], np.float32)
    v = np.asarray(inputs["v"], np.float32)
    wqT = np.ascontiguousarray(np.asarray(inputs["Wq"], np.float32).T).astype(NPF16)
    wkT = np.ascontiguousarray(np.asarray(inputs["Wk"], np.float32).T).astype(NPF16)
    wvT = np.ascontiguousarray(np.asarray(inputs["Wv"], np.float32).T).astype(NPF16)
    woT = np.ascontiguousarray(np.asarray(inputs["Wo"], np.float32).T).astype(NPF16)

    def tslice(x, c):
        # (B, TC, DM) -> (DM, B, TC) -> (DM, R) with r = b*256 + t
        s = x[:, c * TC:(c + 1) * TC, :].transpose(2, 0, 1)
        return np.ascontiguousarray(s).reshape(DM, R).astype(NPF16)

    in_maps = []
    for c in range(N_CORES):
        in_maps.append({
            "qT": tslice(q, c),
            "kT": tslice(k, c),
            "vT": tslice(v, c),
            "wqT": wqT, "wkT": wkT, "wvT": wvT, "woT": woT,
        })
    return in_maps


def _unshard(outs):
    full = np.empty((B, T, DM), np.float32)
    for c in range(N_CORES):
        # out_c[r, m] with r = b*256 + t  ->  (b, t, m)
        full[:, c * TC:(c + 1) * TC, :] = outs[c].reshape(B, TC, DM)
    return full


def run(inputs, trace=False, trace_cores=None, **kw):
    nc = _build()
    in_maps = _shard_inputs(inputs)
    res = run_bass_kernel_spmd(
        nc, in_maps, list(range(N_CORES)),
        trace=trace, trace_cores=trace_cores, **kw)
    return _unshard([m["out"] for m in res.results]), res


def kernel(**inputs):
    out, _ = run(inputs)
    return out



# revision 7
# speedup vs baseline: 1.0204x; 1.0204x over previous
"""AFT-Full attention kernel for 8 TRN2 NeuronCores.

Math: the reference's exp_pos_bias = exp(pos_bias - max(pos_bias, axis=0)) is
identically 1.0 (the max is over a singleton dim), so the two (b,Ti,Tj,Dh)
einsums collapse to per-(b,h) sums over j:
    num[b,h] = sum_j exp(K-max_b K)[b,j,h] * V[b,j,h]
    den[b,h] = sum_j exp(K-max_b K)[b,j,h]
    out = (sigmoid(Q) * num/den) @ Wo.T

Sharding: sequence-parallel over T (256 positions per core, all 4 batches),
weights replicated.  Per-core row index r = b*256 + t_local (b-major) so the
per-(b,h) sums over t are contiguous innermost reduces.  One 32 KiB AllReduce
combines the per-core partial num/den; a 4-byte dummy AllReduce issued at
kernel start absorbs the ~60us one-time collective channel setup so the real
one (~12us) hides behind the Q projection.
"""

import numpy as np
import ml_dtypes

import concourse.bass as bass
import concourse.mybir as mybir
import concourse.tile as tile
from concourse import bacc
from concourse.bass_utils import run_bass_kernel_spmd

B, T, DM, DH = 4, 2048, 1024, 1024
N_CORES = 8
TC = T // N_CORES          # 256 sequence positions per core
R = B * TC                 # 1024 rows per core, r = b*256 + t
P = 128
MC = DM // P               # 8 contraction chunks (d_model)
HB = DH // P               # 8 hidden blocks
NB = 512                   # matmul moving free dim
RB = R // NB               # 2 row blocks
MB = DM // NB              # 2 output-model blocks

F16 = mybir.dt.bfloat16
F32 = mybir.dt.float32
NPF16 = ml_dtypes.bfloat16

_GRAPH = None


def _body(nc, tc):
    qT = nc.dram_tensor("qT", [DM, R], F16, kind="ExternalInput").ap()
    kT = nc.dram_tensor("kT", [DM, R], F16, kind="ExternalInput").ap()
    vT = nc.dram_tensor("vT", [DM, R], F16, kind="ExternalInput").ap()
    wqT = nc.dram_tensor("wqT", [DM, DH], F16, kind="ExternalInput").ap()
    wkT = nc.dram_tensor("wkT", [DM, DH], F16, kind="ExternalInput").ap()
    wvT = nc.dram_tensor("wvT", [DM, DH], F16, kind="ExternalInput").ap()
    woT = nc.dram_tensor("woT", [DH, DM], F16, kind="ExternalInput").ap()
    out = nc.dram_tensor("out", [R, DM], F32, kind="ExternalOutput").ap()

    Exp = mybir.ActivationFunctionType.Exp
    Sigmoid = mybir.ActivationFunctionType.Sigmoid
    Op = mybir.AluOpType

    from contextlib import ExitStack
    with ExitStack() as ctx:
        acts = ctx.enter_context(tc.tile_pool(name="acts", bufs=1))
        work = ctx.enter_context(tc.tile_pool(name="work", bufs=2))
        sqp = ctx.enter_context(tc.tile_pool(name="sqp", bufs=1))
        psum = ctx.enter_context(tc.tile_pool(name="psum", bufs=2, space="PSUM"))
        dram = ctx.enter_context(tc.tile_pool(name="dram", bufs=1, space="DRAM"))

        # Dummy 4-byte AllReduce triggered as the FIRST thing in the kernel:
        # the first collective pays a ~50-120us one-time channel setup +
        # cross-core rendezvous (measured with a collective-only kernel), so
        # the doorbell must ring as early as possible.  It is fed by a tiny
        # memset, NOT by the warmup matmuls, so it does not wait on them.
        dummy = acts.tile([1, 1], F32, name="dummy")
        nc.gpsimd.memset(dummy[:], 0.0)
        d_in = dram.tile([1, 1], F32, name="d_in")
        d_out = dram.tile([1, 1], F32, name="d_out", addr_space="Shared")
        nc.sync.dma_start(d_in[:], dummy[:])
        nc.gpsimd.collective_compute(
            "AllReduce", mybir.AluOpType.add,
            replica_groups=[list(range(N_CORES))],
            ins=[d_in.opt()], outs=[d_out.opt()],
        )
        # Warmup matmuls on zeros un-throttle the PE clock (HAM) while the
        # first input chunks stream in; a copy of their result is stored to
        # a DRAM scratch tile so they are not dead code.
        warm = acts.tile([P, 640], F16, name="warm")
        nc.gpsimd.memset(warm[:], 0.0)
        pwu = psum.tile([P, NB], F32, name="pwu", tag="pk")
        pwu2 = psum.tile([P, NB], F32, name="pwu2", tag="pv")
        for i in range(16):
            t = pwu if i % 2 == 0 else pwu2
            nc.tensor.matmul(t[:], warm[:, 0:P], warm[:, P:P + NB],
                             start=True, stop=True)
        wdump = acts.tile([1, 2], F32, name="wdump")
        nc.vector.tensor_copy(wdump[:, 0:1], pwu[0:1, 0:1])
        nc.vector.tensor_copy(wdump[:, 1:2], pwu2[0:1, 0:1])
        wscratch = dram.tile([1, 2], F32, name="wscratch")
        nc.sync.dma_start(wscratch[:], wdump[:])

        def declare(name, free):
            return acts.tile([P, MC, free], F16, name=name)

        def load_chunk(t, ap_dram, mc):
            src = ap_dram.rearrange("(c p) f -> p c f", p=P)
            nc.sync.dma_start(t[:, mc, :], src[:, mc, :])

        def load(ap_dram, name, free):
            t = declare(name, free)
            nc.sync.dma_start(t[:], ap_dram.rearrange("(c p) f -> p c f", p=P))
            return t

        # K/V operands stream in per-mc so the first matmuls start early
        # (sync ring drains in order).
        kt = declare("kt", R)
        wk = declare("wk", DH)
        for mc in range(MC):
            load_chunk(wk, wkT, mc)
            load_chunk(kt, kT, mc)
        vt = declare("vt", R)
        wv = declare("wv", DH)
        for mc in range(MC):
            load_chunk(wv, wvT, mc)
            load_chunk(vt, vT, mc)
        qt = load(qT, "qt", R)
        wq = load(wqT, "wq", DH)
        wo = load(woT, "wo", DM)

        nd_all = acts.tile([P, HB, 8], F32, name="nd_all")

        # ---- K/V projections + exp + partial num/den ----
        # K runs one hb ahead of V: kt/wk arrive first, and the PE clock is
        # un-throttled only for the first ~24us, so front-load K matmuls.
        def kproj(hb):
            hs = slice(hb * P, (hb + 1) * P)
            pk = psum.tile([P, R], F32, name="pk", tag="pk")
            for mc in range(MC):
                for rb in range(RB):
                    nc.tensor.matmul(
                        pk[:, rb * NB:(rb + 1) * NB],
                        wk[:, mc, hs],
                        kt[:, mc, rb * NB:(rb + 1) * NB],
                        start=(mc == 0), stop=(mc == MC - 1),
                    )
            return pk

        pk_next = kproj(0)
        for hb in range(HB):
            hs = slice(hb * P, (hb + 1) * P)
            pk = pk_next
            if hb + 1 < HB:
                pk_next = kproj(hb + 1)
            pv = psum.tile([P, R], F32, name="pv", tag="pv")
            for mc in range(MC):
                for rb in range(RB):
                    nc.tensor.matmul(
                        pv[:, rb * NB:(rb + 1) * NB],
                        wv[:, mc, hs],
                        vt[:, mc, rb * NB:(rb + 1) * NB],
                        start=(mc == 0), stop=(mc == MC - 1),
                    )
            # max over batch: copy + 3 maxes (DVE may read PSUM only once/op)
            mk = work.tile([P, TC], F32, name="mk")
            nc.vector.tensor_copy(mk[:], pk[:, 0 * TC:1 * TC])
            for b in range(1, B):
                nc.vector.tensor_tensor(
                    mk[:], mk[:], pk[:, b * TC:(b + 1) * TC], op=Op.max)
            ek = work.tile([P, R], F32, name="ek")
            ek3 = ek.rearrange("p (b t) -> p b t", t=TC)
            nc.vector.tensor_tensor(
                ek3, pk.rearrange("p (b t) -> p b t", t=TC),
                mk[:, None, :].to_broadcast((P, B, TC)), op=Op.subtract)
            # exp on ACT with fused per-b den accumulation (contiguous slices)
            for b in range(B):
                bs = slice(b * TC, (b + 1) * TC)
                nc.scalar.activation(
                    ek[:, bs], ek[:, bs], Exp,
                    accum_out=nd_all[:, hb, b:b + 1])
            ekv = work.tile([P, R], F32, name="ekv")
            nc.vector.tensor_tensor(ekv[:], ek[:], pv[:], op=Op.mult)
            nc.vector.tensor_reduce(
                nd_all[:, hb, B:2 * B], ekv.rearrange("p (b t) -> p b t", t=TC),
                axis=mybir.AxisListType.X, op=Op.add)

        # ---- AllReduce of partial num/den, split in two halves so the
        # first can run while the second half of K/V is still computing and
        # the O-phase can start as soon as the first half's r arrives ----
        HH = HB // 2
        nd_sum = []
        for h in range(2):
            red_in = dram.tile([P, HH * 8], F32, name=f"red_in{h}")
            red_out = dram.tile([P, HH * 8], F32, name=f"red_out{h}",
                                addr_space="Shared")
            nc.sync.dma_start(red_in[:], nd_all[:, h * HH:(h + 1) * HH, :])
            nc.gpsimd.collective_compute(
                "AllReduce", Op.add,
                replica_groups=[list(range(N_CORES))],
                ins=[red_in.opt()], outs=[red_out.opt()],
            )
            ns = acts.tile([P, HH, 8], F32, name=f"nd_sum{h}")
            nc.sync.dma_start(ns[:], red_out[:])
            nd_sum.append(ns)

        # ---- Q projection + sigmoid (overlaps the collectives) ----
        sq = []
        for hb in range(HB):
            hs = slice(hb * P, (hb + 1) * P)
            pq = psum.tile([P, R], F32, name="pq", tag="pk")
            for mc in range(MC):
                for rb in range(RB):
                    nc.tensor.matmul(
                        pq[:, rb * NB:(rb + 1) * NB],
                        wq[:, mc, hs],
                        qt[:, mc, rb * NB:(rb + 1) * NB],
                        start=(mc == 0), stop=(mc == MC - 1),
                    )
            s = sqp.tile([P, R], F16, name=f"sq{hb}")
            nc.scalar.activation(s[:], pq[:], Sigmoid)
            sq.append(s)

        # ---- r = num/den per half ----
        r_bf = []
        for h in range(2):
            rden = acts.tile([P, HH, B], F32, name=f"rden{h}")
            nc.vector.reciprocal(rden[:], nd_sum[h][:, :, 0:B])
            rb_ = acts.tile([P, HH, B], F16, name=f"r_bf{h}")
            nc.vector.tensor_tensor(rb_[:], nd_sum[h][:, :, B:2 * B], rden[:],
                                    op=Op.mult)
            r_bf.append(rb_)

        def fold(hc):
            # yt[hc] = sigmoid(Q)[hc] * r[hc]  (in place on sq)
            s3 = sq[hc].rearrange("p (b t) -> p b t", t=TC)
            nc.vector.tensor_tensor(
                s3, s3,
                r_bf[hc // HH][:, hc % HH, :, None].to_broadcast((P, B, TC)),
                op=Op.mult)

        # ---- output projection: hc-outer so each step needs only r[hc];
        # two passes of 4 row-blocks each (4 x [128,1024] fp32 = all 8 PSUM
        # banks per pass) ----
        for p2 in range(2):
            pos = [psum.tile([P, DM], F32, name=f"po{p2}_{rb4}",
                             tag=("pk" if rb4 < 2 else "pv"))
                   for rb4 in range(4)]
            for hc in range(HB):
                if p2 == 0:
                    fold(hc)
                for rb4 in range(4):
                    rblk = p2 * 4 + rb4
                    rs = slice(rblk * P, (rblk + 1) * P)
                    for mb in range(MB):
                        nc.tensor.matmul(
                            pos[rb4][:, mb * NB:(mb + 1) * NB],
                            sq[hc][:, rs],
                            wo[:, hc, mb * NB:(mb + 1) * NB],
                            start=(hc == 0), stop=(hc == HB - 1),
                        )
            for rb4 in range(4):
                rblk = p2 * 4 + rb4
                rs = slice(rblk * P, (rblk + 1) * P)
                ot = work.tile([P, DM], F32, name="ot")
                if rb4 % 2 == 0:
                    nc.vector.tensor_copy(ot[:], pos[rb4][:])
                else:
                    nc.scalar.copy(ot[:], pos[rb4][:])
                nc.sync.dma_start(out[rs, :], ot[:])


def _dedup_ldweights(nc):
    """Drop InstLdweights whose weight AP is identical to the PE's already-
    loaded weights (the 2nd matmul of each rb-pair reloads the same tile).
    Each reload costs ~50ns of weight-plane fill serialized into the next
    matmul (263ns vs 216ns per MM measured), so halving LDW count saves
    ~12-25us across 512 matmuls.  Waits/updates of a dropped LDW are merged
    into the following PE instruction (fires later -> still safe)."""
    PE = mybir.EngineType.PE
    for f in nc.m.functions:
        for blk in f.blocks:
            insts = list(blk.instructions)
            keep = []
            last_sig = None
            pending_si = None
            for inst in insts:
                if inst.engine == PE:
                    if isinstance(inst, mybir.InstLdweights):
                        sig = (str(inst.ins[0]), str(inst.tile_position),
                               str(inst.perf_mode), str(inst.is_transpose))
                        if sig == last_sig:
                            si = inst.sync_info
                            if si is not None and (si.on_wait or si.on_update):
                                if pending_si is None:
                                    pending_si = ([], [])
                                pending_si[0].extend(si.on_wait)
                                pending_si[1].extend(si.on_update)
                            continue  # drop redundant reload
                        last_sig = sig
                    elif isinstance(inst, mybir.InstMatmult):
                        if pending_si is not None:
                            si = inst.sync_info
                            if si is None:
                                si = mybir.SyncInfo(on_wait=[], on_update=[])
                            inst.sync_info = mybir.SyncInfo(
                                on_wait=list(si.on_wait) + pending_si[0],
                                on_update=list(si.on_update) + pending_si[1],
                            )
                            pending_si = None
                    elif isinstance(inst, (mybir.InstEventSemaphore,
                                           mybir.InstNoOp, mybir.InstDrain)):
                        pass  # sequencer-only ops don't touch the PE array
                    else:
                        last_sig = None  # unknown PE op: be conservative
                keep.append(inst)
            assert pending_si is None
            if len(keep) != len(insts):
                blk.instructions[:] = keep


def _build():
    global _GRAPH
    if _GRAPH is None:
        nc = bacc.Bacc("TRN2", target_bir_lowering=False, debug=False,
                       num_devices=N_CORES)
        with tile.TileContext(nc) as tc:
            _body(nc, tc)
        _dedup_ldweights(nc)
        nc.compile()
        _GRAPH = nc
    return _GRAPH


def _shard_inputs(inputs):
    q = np.asarray(inputs["q"], np.float32)
    k = np.asarray(inputs["k"], np.float32)
    v = np.asarray(inputs["v"], np.float32)
    wqT = np.ascontiguousarray(np.asarray(inputs["Wq"], np.float32).T).astype(NPF16)
    wkT = np.ascontiguousarray(np.asarray(inputs["Wk"], np.float32).T).astype(NPF16)
    wvT = np.ascontiguousarray(np.asarray(inputs["Wv"], np.float32).T).astype(NPF16)
    woT = np.ascontiguousarray(np.asarray(inputs["Wo"], np.float32).T).astype(NPF16)

    def tslice(x, c):
        # (B, TC, DM) -> (DM, B, TC) -> (DM, R) with r = b*256 + t
        s = x[:, c * TC:(c + 1) * TC, :].transpose(2, 0, 1)
        return np.ascontiguousarray(s).reshape(DM, R).astype(NPF16)

    in_maps = []
    for c in range(N_CORES):
        in_maps.append({
            "qT": tslice(q, c),
            "kT": tslice(k, c),
            "vT": tslice(v, c),
            "wqT": wqT, "wkT": wkT, "wvT": wvT, "woT": woT,
        })
    return in_maps


def _unshard(outs):
    full = np.empty((B, T, DM), np.float32)
    for c in range(N_CORES):
        # out_c[r, m] with r = b*256 + t  ->  (b, t, m)
        full[:, c * TC:(c + 1) * TC, :] = outs[c].reshape(B, TC, DM)
    return full


def run(inputs, trace=False, trace_cores=None, **kw):
    nc = _build()
    in_maps = _shard_inputs(inputs)
    res = run_bass_kernel_spmd(
        nc, in_maps, list(range(N_CORES)),
        trace=trace, trace_cores=trace_cores, **kw)
    return _unshard([m["out"] for m in res.results]), res


def kernel(**inputs):
    out, _ = run(inputs)
    return out



# revision 14
# speedup vs baseline: 1.0570x; 1.0358x over previous
"""AFT-Full attention kernel for 8 TRN2 NeuronCores.

Math: the reference's exp_pos_bias = exp(pos_bias - max(pos_bias, axis=0)) is
identically 1.0 (the max is over a singleton dim), so the two (b,Ti,Tj,Dh)
einsums collapse to per-(b,h) sums over j:
    num[b,h] = sum_j exp(K-max_b K)[b,j,h] * V[b,j,h]
    den[b,h] = sum_j exp(K-max_b K)[b,j,h]
    out = (sigmoid(Q) * num/den) @ Wo.T

Sharding: sequence-parallel over T (256 positions per core, all 4 batches),
weights replicated.  Per-core row index r = b*256 + t_local (b-major) so the
per-(b,h) sums over t are contiguous innermost reduces.  One 32 KiB AllReduce
combines the per-core partial num/den; a 4-byte dummy AllReduce issued at
kernel start absorbs the ~60us one-time collective channel setup so the real
one (~12us) hides behind the Q projection.
"""

import numpy as np
import ml_dtypes

import concourse.bass as bass
import concourse.mybir as mybir
import concourse.tile as tile
from concourse import bacc
from concourse.bass_utils import run_bass_kernel_spmd

B, T, DM, DH = 4, 2048, 1024, 1024
N_CORES = 8
TC = T // N_CORES          # 256 sequence positions per core
R = B * TC                 # 1024 rows per core, r = b*256 + t
P = 128
MC = DM // P               # 8 contraction chunks (d_model)
HB = DH // P               # 8 hidden blocks
NB = 512                   # matmul moving free dim
RB = R // NB               # 2 row blocks
MB = DM // NB              # 2 output-model blocks

F16 = mybir.dt.bfloat16
F8 = mybir.dt.float8e4
F32 = mybir.dt.float32
NPF16 = ml_dtypes.bfloat16
NPF8 = ml_dtypes.float8_e4m3   # TRN FP8_EXP4: max +-240, matches in range
# fp8 pre-scales for the Q path (descaled inside the sigmoid activation).
# q ~ N(0,1) * 16 -> +-88 max; Wq ~ N(0,0.02^2) * 512 -> +-56 max: no clip,
# negligible subnormals.  Sigmoid damps the fp8 quantization 4x; simulated
# end-to-end rel err 1.07e-2 vs the 2e-2 gate.
QSCALE = 16.0
WQSCALE = 512.0

_GRAPH = None


def _body(nc, tc):
    qT = nc.dram_tensor("qT", [DM, R], F8, kind="ExternalInput").ap()
    kT = nc.dram_tensor("kT", [DM, R], F16, kind="ExternalInput").ap()
    vT = nc.dram_tensor("vT", [DM, R], F16, kind="ExternalInput").ap()
    wqT = nc.dram_tensor("wqT", [DM, DH], F8, kind="ExternalInput").ap()
    wkT = nc.dram_tensor("wkT", [DM, DH], F16, kind="ExternalInput").ap()
    wvT = nc.dram_tensor("wvT", [DM, DH], F16, kind="ExternalInput").ap()
    woT = nc.dram_tensor("woT", [DH, DM], F16, kind="ExternalInput").ap()
    out = nc.dram_tensor("out", [R, DM], F32, kind="ExternalOutput").ap()

    Exp = mybir.ActivationFunctionType.Exp
    Sigmoid = mybir.ActivationFunctionType.Sigmoid
    Op = mybir.AluOpType

    from contextlib import ExitStack
    with ExitStack() as ctx:
        acts = ctx.enter_context(tc.tile_pool(name="acts", bufs=1))
        work = ctx.enter_context(tc.tile_pool(name="work", bufs=2))
        sqp = ctx.enter_context(tc.tile_pool(name="sqp", bufs=1))
        psum = ctx.enter_context(tc.tile_pool(name="psum", bufs=2, space="PSUM"))
        dram = ctx.enter_context(tc.tile_pool(name="dram", bufs=1, space="DRAM"))

        # Dummy 4-byte AllReduce triggered as the FIRST thing in the kernel:
        # the first collective pays a ~50-120us one-time channel setup +
        # cross-core rendezvous (measured with a collective-only kernel), so
        # the doorbell must ring as early as possible.  It is fed by a tiny
        # memset, NOT by the warmup matmuls, so it does not wait on them.
        dummy = acts.tile([1, 1], F32, name="dummy")
        nc.gpsimd.memset(dummy[:], 0.0)
        d_in = dram.tile([1, 1], F32, name="d_in")
        d_out = dram.tile([1, 1], F32, name="d_out", addr_space="Shared")
        # scalar-queue HWDGE so the sync queue's first ops stay the input
        # loads (the K matmuls are gated on those)
        nc.scalar.dma_start(d_in[:], dummy[:])
        nc.gpsimd.collective_compute(
            "AllReduce", mybir.AluOpType.add,
            replica_groups=[list(range(N_CORES))],
            ins=[d_in.opt()], outs=[d_out.opt()],
        )
        # Warmup matmuls on zeros un-throttle the PE clock (HAM) while the
        # first input chunks stream in; a copy of their result is stored to
        # a DRAM scratch tile so they are not dead code.
        warm = acts.tile([P, 640], F16, name="warm")
        nc.gpsimd.memset(warm[:], 0.0)
        pwu = psum.tile([P, NB], F32, name="pwu", tag="pk")
        pwu2 = psum.tile([P, NB], F32, name="pwu2", tag="pv")
        for i in range(16):
            t = pwu if i % 2 == 0 else pwu2
            nc.tensor.matmul(t[:], warm[:, 0:P], warm[:, P:P + NB],
                             start=True, stop=True)
        wdump = acts.tile([1, 2], F32, name="wdump")
        nc.vector.tensor_copy(wdump[:, 0:1], pwu[0:1, 0:1])
        nc.vector.tensor_copy(wdump[:, 1:2], pwu2[0:1, 0:1])
        wscratch = dram.tile([1, 2], F32, name="wscratch")
        nc.sync.dma_start(wscratch[:], wdump[:])

        def declare(name, free):
            return acts.tile([P, MC, free], F16, name=name)

        def load_chunk(t, ap_dram, mc):
            src = ap_dram.rearrange("(c p) f -> p c f", p=P)
            nc.sync.dma_start(t[:, mc, :], src[:, mc, :])

        def load(ap_dram, name, free):
            t = declare(name, free)
            nc.sync.dma_start(t[:], ap_dram.rearrange("(c p) f -> p c f", p=P))
            return t

        # K/V operands stream in per-mc so the first matmuls start early
        # (sync ring drains in order).
        kt = declare("kt", R)
        wk = declare("wk", DH)
        for mc in range(MC):
            load_chunk(wk, wkT, mc)
            load_chunk(kt, kT, mc)
        vt = declare("vt", R)
        wv = declare("wv", DH)
        for mc in range(MC):
            load_chunk(wv, wvT, mc)
            load_chunk(vt, vT, mc)
        qt = acts.tile([P, MC, R], F8, name="qt")
        nc.sync.dma_start(qt[:], qT.rearrange("(c p) f -> p c f", p=P))
        wq = acts.tile([P, MC, DH], F8, name="wq")
        nc.sync.dma_start(wq[:], wqT.rearrange("(c p) f -> p c f", p=P))
        wo = load(woT, "wo", DM)

        nd_all = acts.tile([P, HB, 8], F32, name="nd_all")

        # ---- K/V projections + exp + partial num/den ----
        # K runs one hb ahead of V: kt/wk arrive first, and the PE clock is
        # un-throttled only for the first ~24us, so front-load K matmuls.
        def kproj(hb):
            hs = slice(hb * P, (hb + 1) * P)
            pk = psum.tile([P, R], F32, name="pk", tag="pk")
            for mc in range(MC):
                for rb in range(RB):
                    nc.tensor.matmul(
                        pk[:, rb * NB:(rb + 1) * NB],
                        wk[:, mc, hs],
                        kt[:, mc, rb * NB:(rb + 1) * NB],
                        start=(mc == 0), stop=(mc == MC - 1),
                    )
            return pk

        pk_next = kproj(0)
        for hb in range(HB):
            hs = slice(hb * P, (hb + 1) * P)
            pk = pk_next
            if hb + 1 < HB:
                pk_next = kproj(hb + 1)
            pv = psum.tile([P, R], F32, name="pv", tag="pv")
            for mc in range(MC):
                for rb in range(RB):
                    nc.tensor.matmul(
                        pv[:, rb * NB:(rb + 1) * NB],
                        wv[:, mc, hs],
                        vt[:, mc, rb * NB:(rb + 1) * NB],
                        start=(mc == 0), stop=(mc == MC - 1),
                    )
            # max over batch: copy + 3 maxes (DVE may read PSUM only once/op)
            mk = work.tile([P, TC], F32, name="mk")
            nc.vector.tensor_copy(mk[:], pk[:, 0 * TC:1 * TC])
            for b in range(1, B):
                nc.vector.tensor_tensor(
                    mk[:], mk[:], pk[:, b * TC:(b + 1) * TC], op=Op.max)
            ek = work.tile([P, R], F32, name="ek")
            ek3 = ek.rearrange("p (b t) -> p b t", t=TC)
            nc.vector.tensor_tensor(
                ek3, pk.rearrange("p (b t) -> p b t", t=TC),
                mk[:, None, :].to_broadcast((P, B, TC)), op=Op.subtract)
            # exp on ACT with fused per-b den accumulation (contiguous slices)
            for b in range(B):
                bs = slice(b * TC, (b + 1) * TC)
                nc.scalar.activation(
                    ek[:, bs], ek[:, bs], Exp,
                    accum_out=nd_all[:, hb, b:b + 1])
            ekv = work.tile([P, R], F32, name="ekv")
            nc.vector.tensor_tensor(ekv[:], ek[:], pv[:], op=Op.mult)
            nc.vector.tensor_reduce(
                nd_all[:, hb, B:2 * B], ekv.rearrange("p (b t) -> p b t", t=TC),
                axis=mybir.AxisListType.X, op=Op.add)

        # ---- AllReduce of partial num/den, split in two halves so the
        # first can run while the second half of K/V is still computing and
        # the O-phase can start as soon as the first half's r arrives ----
        HH = HB // 2
        nd_sum = []
        for h in range(2):
            red_in = dram.tile([P, HH * 8], F32, name=f"red_in{h}")
            red_out = dram.tile([P, HH * 8], F32, name=f"red_out{h}",
                                addr_space="Shared")
            nc.sync.dma_start(red_in[:], nd_all[:, h * HH:(h + 1) * HH, :])
            nc.gpsimd.collective_compute(
                "AllReduce", Op.add,
                replica_groups=[list(range(N_CORES))],
                ins=[red_in.opt()], outs=[red_out.opt()],
            )
            ns = acts.tile([P, HH, 8], F32, name=f"nd_sum{h}")
            nc.sync.dma_start(ns[:], red_out[:])
            nd_sum.append(ns)

        # ---- Q projection (fp8 DoubleRow: 2 k-chunks per matmul, halves
        # the PE instruction count) + sigmoid with the fp8 descale folded ----
        DR = mybir.MatmulPerfMode.DoubleRow
        sq = []
        for hb in range(HB):
            hs = slice(hb * P, (hb + 1) * P)
            pq = psum.tile([P, R], F32, name="pq", tag="pk")
            for mc2 in range(0, MC, 2):
                for rb in range(RB):
                    nc.tensor.matmul(
                        pq[:, rb * NB:(rb + 1) * NB],
                        wq[:, mc2:mc2 + 2, hs],
                        qt[:, mc2:mc2 + 2, rb * NB:(rb + 1) * NB],
                        start=(mc2 == 0), stop=(mc2 == MC - 2),
                        perf_mode=DR,
                    )
            s = sqp.tile([P, R], F16, name=f"sq{hb}")
            nc.scalar.activation(s[:], pq[:], Sigmoid,
                                 scale=1.0 / (QSCALE * WQSCALE))
            sq.append(s)

        # ---- r = num/den per half ----
        r_bf = []
        for h in range(2):
            rden = acts.tile([P, HH, B], F32, name=f"rden{h}")
            nc.vector.reciprocal(rden[:], nd_sum[h][:, :, 0:B])
            rb_ = acts.tile([P, HH, B], F16, name=f"r_bf{h}")
            nc.vector.tensor_tensor(rb_[:], nd_sum[h][:, :, B:2 * B], rden[:],
                                    op=Op.mult)
            r_bf.append(rb_)

        def fold(hc):
            # yt[hc] = sigmoid(Q)[hc] * r[hc]  (in place on sq)
            s3 = sq[hc].rearrange("p (b t) -> p b t", t=TC)
            nc.vector.tensor_tensor(
                s3, s3,
                r_bf[hc // HH][:, hc % HH, :, None].to_broadcast((P, B, TC)),
                op=Op.mult)

        # ---- output projection.  Pass A (row-blocks 0-3) is hc-outer so
        # each accumulation step needs only r[hc] -- the O matmuls start as
        # soon as the first AllReduce half lands, tolerating a late second
        # half.  Pass B (row-blocks 4-7) runs rblk-outer (all r available by
        # then) so its copies/DMAs pipeline with the remaining matmuls. ----
        pos = [psum.tile([P, DM], F32, name=f"poA{rb4}",
                         tag=("pk" if rb4 < 2 else "pv"))
               for rb4 in range(4)]
        for hc in range(HB):
            fold(hc)
            for rb4 in range(4):
                rs = slice(rb4 * P, (rb4 + 1) * P)
                for mb in range(MB):
                    nc.tensor.matmul(
                        pos[rb4][:, mb * NB:(mb + 1) * NB],
                        sq[hc][:, rs],
                        wo[:, hc, mb * NB:(mb + 1) * NB],
                        start=(hc == 0), stop=(hc == HB - 1),
                    )
        for rb4 in range(4):
            rs = slice(rb4 * P, (rb4 + 1) * P)
            ot = work.tile([P, DM], F32, name="ot")
            if rb4 % 2 == 0:
                nc.vector.tensor_copy(ot[:], pos[rb4][:])
            else:
                nc.scalar.copy(ot[:], pos[rb4][:])
            nc.sync.dma_start(out[rs, :], ot[:])
        for rblk in range(4, HB):
            rs = slice(rblk * P, (rblk + 1) * P)
            po = psum.tile([P, DM], F32, name="poB",
                           tag=("pk" if rblk % 2 == 0 else "pv"))
            for hc in range(HB):
                for mb in range(MB):
                    nc.tensor.matmul(
                        po[:, mb * NB:(mb + 1) * NB],
                        sq[hc][:, rs],
                        wo[:, hc, mb * NB:(mb + 1) * NB],
                        start=(hc == 0), stop=(hc == HB - 1),
                    )
            ot = work.tile([P, DM], F32, name="ot")
            if rblk == HB - 1:
                nc.vector.tensor_copy(ot[:, 0:NB], po[:, 0:NB])
                nc.sync.dma_start(out[rs, 0:NB], ot[:, 0:NB])
                nc.scalar.copy(ot[:, NB:DM], po[:, NB:DM])
                nc.sync.dma_start(out[rs, NB:DM], ot[:, NB:DM])
            else:
                if rblk % 2 == 0:
                    nc.vector.tensor_copy(ot[:], po[:])
                else:
                    nc.scalar.copy(ot[:], po[:])
                nc.sync.dma_start(out[rs, :], ot[:])


def _dedup_ldweights(nc):
    """Drop InstLdweights whose weight AP is identical to the PE's already-
    loaded weights (the 2nd matmul of each rb-pair reloads the same tile).
    Each reload costs ~50ns of weight-plane fill serialized into the next
    matmul (263ns vs 216ns per MM measured), so halving LDW count saves
    ~12-25us across 512 matmuls.  Waits/updates of a dropped LDW are merged
    into the following PE instruction (fires later -> still safe)."""
    PE = mybir.EngineType.PE
    for f in nc.m.functions:
        for blk in f.blocks:
            insts = list(blk.instructions)
            keep = []
            last_sig = None
            pending_si = None
            for inst in insts:
                if inst.engine == PE:
                    if isinstance(inst, mybir.InstLdweights):
                        sig = (str(inst.ins[0]), str(inst.tile_position),
                               str(inst.perf_mode), str(inst.is_transpose))
                        if sig == last_sig:
                            si = inst.sync_info
                            if si is not None and (si.on_wait or si.on_update):
                                if pending_si is None:
                                    pending_si = ([], [])
                                pending_si[0].extend(si.on_wait)
                                pending_si[1].extend(si.on_update)
                            continue  # drop redundant reload
                        last_sig = sig
                    elif isinstance(inst, mybir.InstMatmult):
                        if pending_si is not None:
                            si = inst.sync_info
                            if si is None:
                                si = mybir.SyncInfo(on_wait=[], on_update=[])
                            inst.sync_info = mybir.SyncInfo(
                                on_wait=list(si.on_wait) + pending_si[0],
                                on_update=list(si.on_update) + pending_si[1],
                            )
                            pending_si = None
                    elif isinstance(inst, (mybir.InstEventSemaphore,
                                           mybir.InstNoOp, mybir.InstDrain)):
                        pass  # sequencer-only ops don't touch the PE array
                    else:
                        last_sig = None  # unknown PE op: be conservative
                keep.append(inst)
            assert pending_si is None
            if len(keep) != len(insts):
                blk.instructions[:] = keep


def _build():
    global _GRAPH
    if _GRAPH is None:
        nc = bacc.Bacc("TRN2", target_bir_lowering=False, debug=False,
                       num_devices=N_CORES)
        with tile.TileContext(nc) as tc:
            _body(nc, tc)
        _dedup_ldweights(nc)
        nc.compile()
        _GRAPH = nc
    return _GRAPH


def _shard_inputs(inputs):
    q = np.asarray(inputs["q"], np.float32)
    k = np.asarray(inputs["k"], np.float32)
    v = np.asarray(inputs["v"], np.float32)
    wqT = np.ascontiguousarray(
        np.asarray(inputs["Wq"], np.float32).T * WQSCALE).astype(NPF8)
    wkT = np.ascontiguousarray(np.asarray(inputs["Wk"], np.float32).T).astype(NPF16)
    wvT = np.ascontiguousarray(np.asarray(inputs["Wv"], np.float32).T).astype(NPF16)
    woT = np.ascontiguousarray(np.asarray(inputs["Wo"], np.float32).T).astype(NPF16)

    def tslice(x, c, dt=NPF16, scale=None):
        # (B, TC, DM) -> (DM, B, TC) -> (DM, R) with r = b*256 + t
        s = x[:, c * TC:(c + 1) * TC, :].transpose(2, 0, 1)
        s = np.ascontiguousarray(s).reshape(DM, R)
        if scale is not None:
            s = s * scale
        return s.astype(dt)

    in_maps = []
    for c in range(N_CORES):
        in_maps.append({
            "qT": tslice(q, c, NPF8, QSCALE),
            "kT": tslice(k, c),
            "vT": tslice(v, c),
            "wqT": wqT, "wkT": wkT, "wvT": wvT, "woT": woT,
        })
    return in_maps


def _unshard(outs):
    full = np.empty((B, T, DM), np.float32)
    for c in range(N_CORES):
        # out_c[r, m] with r = b*256 + t  ->  (b, t, m)
        full[:, c * TC:(c + 1) * TC, :] = outs[c].reshape(B, TC, DM)
    return full


def run(inputs, trace=False, trace_cores=None, **kw):
    nc = _build()
    in_maps = _shard_inputs(inputs)
    res = run_bass_kernel_spmd(
        nc, in_maps, list(range(N_CORES)),
        trace=trace, trace_cores=trace_cores, **kw)
    return _unshard([m["out"] for m in res.results]), res


def kernel(**inputs):
    out, _ = run(inputs)
    return out

